# revision 1
# baseline (speedup 1.0000x reference)
"""Trainium2 Bass kernel for nn_Attention_42279658062045 (gnn_message_passing).

Computes, for each of B=200000 nodes:
    simi   = exp(-source_distance^2 / 2)                  [B, K]
    weight = softmax(simi @ kernel + bias, axis=-1)       [B, K]
    mean   = einsum('bk,bkd->bd', weight, context)        [B, D]

Sharding: pure data parallel over the node axis B across 8 NeuronCores;
kernel/bias replicated; no cross-device communication.

Per-core dataflow (B_LOCAL = 25000 rows, tiles of 128 rows):
  - HWDGE streams context in 2-tile (~2 MB) chunks (dominant HBM traffic).
  - ACT: square+exp of the distances (batched over all tiles), PSUM->SBUF
    copies, exp(logits) with accum_out giving the softmax denominator.
  - PE: transpose of simi tiles, simi @ kernel (+ bias via a second
    accumulating matmul with a ones-row stationary).
  - DVE: reciprocal of the denominator, fused (ctx * (1/Z)) * exp(logits)
    product (scalar_tensor_tensor), and the k-reduction (strided reduce_sum).
"""

import numpy as np

N_CORES = 8
B, K, D = 200000, 30, 64
B_LOCAL = B // N_CORES  # 25000
P = 128
CT = 2  # tiles per context DMA chunk

_CACHE = {}


def _build():
    import concourse.bacc as bacc
    import concourse.tile as tile
    from concourse import mybir
    from concourse.masks import make_identity

    fp32 = mybir.dt.float32
    AF = mybir.ActivationFunctionType

    nc = bacc.Bacc("TRN2", target_bir_lowering=False, debug=False,
                   num_devices=N_CORES)

    dist = nc.dram_tensor("source_distance", [B_LOCAL, K], fp32,
                          kind="ExternalInput").ap()
    ctx_d = nc.dram_tensor("context", [B_LOCAL, K, D], fp32,
                           kind="ExternalInput").ap()
    kern = nc.dram_tensor("kernel", [K, K], fp32, kind="ExternalInput").ap()
    bias = nc.dram_tensor("bias", [K], fp32, kind="ExternalInput").ap()
    out = nc.dram_tensor("out", [B_LOCAL, D], fp32, kind="ExternalOutput").ap()

    n_full = B_LOCAL // P          # 195 full tiles
    rem = B_LOCAL - n_full * P     # 40 leftover rows

    # [128, n_full, ...] views: partition = row-within-tile, dim1 = tile idx
    dist_v = dist[:n_full * P, :].rearrange("(n p) k -> p n k", p=P)
    ctx_v = ctx_d[:n_full * P].rearrange("(n p) k d -> p n (k d)", p=P)
    out_v = out[:n_full * P, :].rearrange("(n p) d -> p n d", p=P)

    with tile.TileContext(nc) as tc:
        from contextlib import ExitStack
        with ExitStack() as st:
            consts = st.enter_context(tc.tile_pool(name="consts", bufs=1))
            big = st.enter_context(tc.tile_pool(name="big", bufs=1))
            ctxp = st.enter_context(tc.tile_pool(name="ctx", bufs=3))
            prodp = st.enter_context(tc.tile_pool(name="prod", bufs=2))
            small = st.enter_context(tc.tile_pool(name="small", bufs=3))
            psum_t = st.enter_context(
                tc.tile_pool(name="psumT", bufs=2, space="PSUM"))
            psum_l = st.enter_context(
                tc.tile_pool(name="psumL", bufs=2, space="PSUM"))

            ident = consts.tile([P, P], fp32)
            make_identity(nc, ident)
            kern_s = consts.tile([K, K], fp32)
            nc.sync.dma_start(out=kern_s, in_=kern)
            bias_s = consts.tile([1, K], fp32)
            nc.sync.dma_start(out=bias_s, in_=bias.unsqueeze(0))
            ones_s = consts.tile([1, P], fp32)
            nc.vector.memset(ones_s, 1.0)

            # All distances for the full tiles; squared+exp'd in place.
            simi_all = big.tile([P, n_full, K], fp32)
            nc.sync.dma_start(out=simi_all, in_=dist_v)
            nc.scalar.activation(out=simi_all, in_=simi_all, func=AF.Square)
            nc.scalar.activation(out=simi_all, in_=simi_all, func=AF.Exp,
                                 scale=-0.5)

            # Staged output for the full tiles (written once at the end).
            mean_all = big.tile([P, n_full, D], fp32)

            def do_tile(simi_ap, ctx_ap, mean_ap, rows):
                """One 128-row (or partial) tile.

                simi_ap: [rows, K] SBUF, ctx_ap: [rows, K*D] SBUF,
                mean_ap: [rows, D] SBUF destination.
                """
                simiT_p = psum_t.tile([K, P], fp32, tag="simiT_p")
                nc.tensor.transpose(out=simiT_p[:, :rows], in_=simi_ap,
                                    identity=ident[:rows, :rows])
                simiT_s = small.tile([K, P], fp32, tag="simiT_s")
                nc.scalar.copy(out=simiT_s[:, :rows], in_=simiT_p[:, :rows])

                logits_p = psum_l.tile([P, K], fp32, tag="logits_p")
                nc.tensor.matmul(out=logits_p[:rows, :],
                                 lhsT=simiT_s[:, :rows], rhs=kern_s,
                                 start=True, stop=False)
                nc.tensor.matmul(out=logits_p[:rows, :],
                                 lhsT=ones_s[:, :rows], rhs=bias_s,
                                 start=False, stop=True)

                expw = small.tile([P, K], fp32, tag="expw")
                zsum = small.tile([P, 1], fp32, tag="zsum")
                nc.scalar.activation(out=expw[:rows, :], in_=logits_p[:rows, :],
                                     func=AF.Exp, accum_out=zsum[:rows, :])
                rz = small.tile([P, 1], fp32, tag="rz")
                nc.vector.reciprocal(out=rz[:rows, :], in_=zsum[:rows, :])

                # prod[p, k, d] = (ctx[p, k, d] * rz[p]) * expw[p, k]
                prod = prodp.tile([P, K, D], fp32, tag="prod")
                ctx3 = ctx_ap.rearrange("p (k d) -> p k d", k=K)
                w_bcast = expw[:rows, :].unsqueeze(2).broadcast_to([rows, K, D])
                nc.vector.scalar_tensor_tensor(
                    out=prod[:rows], in0=ctx3, scalar=rz[:rows, :],
                    in1=w_bcast, op0=mybir.AluOpType.mult,
                    op1=mybir.AluOpType.mult)

                # mean[p, d] = sum_k prod[p, k, d]
                nc.vector.reduce_sum(out=mean_ap,
                                     in_=prod[:rows].rearrange("p k d -> p d k"),
                                     axis=mybir.AxisListType.X)

            # Full tiles, context DMA'd in CT-tile chunks.
            for c0 in range(0, n_full, CT):
                cn = min(CT, n_full - c0)
                ctx_tile = ctxp.tile([P, CT, K * D], fp32, tag="ctx")
                nc.sync.dma_start(out=ctx_tile[:, :cn, :],
                                  in_=ctx_v[:, c0:c0 + cn, :])
                for j in range(cn):
                    t = c0 + j
                    do_tile(simi_all[:, t, :], ctx_tile[:, j, :],
                            mean_all[:, t, :], P)

            nc.sync.dma_start(out=out_v, in_=mean_all)

            # Remainder rows (partial tile).
            if rem:
                simi_r = small.tile([P, K], fp32, tag="simi_r")
                nc.sync.dma_start(out=simi_r[:rem, :], in_=dist[n_full * P:, :])
                nc.scalar.activation(out=simi_r[:rem, :], in_=simi_r[:rem, :],
                                     func=AF.Square)
                nc.scalar.activation(out=simi_r[:rem, :], in_=simi_r[:rem, :],
                                     func=AF.Exp, scale=-0.5)
                ctx_r = ctxp.tile([P, CT, K * D], fp32, tag="ctx")
                nc.sync.dma_start(
                    out=ctx_r[:rem, 0, :],
                    in_=ctx_d[n_full * P:].rearrange("b k d -> b (k d)"))
                mean_r = small.tile([P, D], fp32, tag="mean_r")
                do_tile(simi_r[:rem, :], ctx_r[:rem, 0, :], mean_r[:rem, :], rem)
                nc.sync.dma_start(out=out[n_full * P:, :], in_=mean_r[:rem, :])

    nc.compile()
    return nc


def _get_nc():
    if "nc" not in _CACHE:
        _CACHE["nc"] = _build()
    return _CACHE["nc"]


def kernel(source_distance, context, kernel, bias, _trace=False, _tmpdir=None):
    from concourse.bass_utils import run_bass_kernel_spmd

    nc = _get_nc()

    source_distance = np.ascontiguousarray(source_distance, dtype=np.float32)
    context = np.ascontiguousarray(context, dtype=np.float32)
    kernel = np.ascontiguousarray(kernel, dtype=np.float32)
    bias = np.ascontiguousarray(bias, dtype=np.float32)

    in_maps = []
    for i in range(N_CORES):
        lo, hi = i * B_LOCAL, (i + 1) * B_LOCAL
        in_maps.append({
            "source_distance": source_distance[lo:hi],
            "context": context[lo:hi],
            "kernel": kernel,
            "bias": bias,
        })

    res = run_bass_kernel_spmd(nc, in_maps, list(range(N_CORES)),
                               trace=_trace, tmpdir=_tmpdir)
    out = np.concatenate([res.results[i]["out"] for i in range(N_CORES)], axis=0)
    if _trace:
        _CACHE["last_results"] = res
    return out


# revision 2
# speedup vs baseline: 1.1377x; 1.1377x over previous
"""Trainium2 Bass kernel for nn_Attention_42279658062045 (gnn_message_passing).

Computes, for each of B=200000 nodes:
    simi   = exp(-source_distance^2 / 2)                  [B, K]
    weight = softmax(simi @ kernel + bias, axis=-1)       [B, K]
    mean   = einsum('bk,bkd->bd', weight, context)        [B, D]

Sharding: pure data parallel over the node axis B across 8 NeuronCores;
kernel/bias replicated; no cross-device communication.

Per-core dataflow (B_LOCAL = 25000 rows, tiles of 128 rows, f32 throughout):
  - HWDGE streams context in 2-tile (~2 MB) chunks (dominant HBM traffic).
  - PE: transpose of simi tiles; logits = simi @ kernel + bias via two
    accumulating matmuls (ones-row stationary adds the bias).
  - ACT: batched square+exp of all distances, PSUM->SBUF copy of simi^T,
    exp(logits) with accum_out giving the softmax denominator, and the
    weighted product for 5 of the 30 k-slabs (per-partition scale).
  - DVE: reciprocal, weight normalize, weighted product for 25 k-slabs,
    and the final k-reduction.
  - GPSIMD: one batched fold (k -> k/2) per 4-tile chunk.

The product tensor uses an interleaved layout [d_hi(32), k(30), d_lo(2)]
(flat addr = d_hi*60 + k*2 + d_lo) so the k-reduction reads at 8-byte
stride, which the DVE streams at full rate (256-byte strides cost ~1.6x).
"""

import numpy as np

N_CORES = 8
B, K, D = 200000, 30, 64
B_LOCAL = B // N_CORES  # 25000
P = 128
CT = 2          # tiles per context DMA chunk
PT = 4          # tiles per product chunk (one GPSIMD fold per chunk)
IL = 2          # product interleave: [d_hi(32), k(30), d_lo(IL)]
DH = D // IL    # 32
KH = K // 2     # 15 (fold halves)
ACT_SLABS = 5   # k-slabs whose product is computed on the scalar engine
DVE_SLABS = K - ACT_SLABS

_CACHE = {}


def _build():
    import concourse.bacc as bacc
    import concourse.tile as tile
    from concourse import mybir
    from concourse.masks import make_identity

    fp32 = mybir.dt.float32
    AF = mybir.ActivationFunctionType

    nc = bacc.Bacc("TRN2", target_bir_lowering=False, debug=False,
                   num_devices=N_CORES)

    dist = nc.dram_tensor("source_distance", [B_LOCAL, K], fp32,
                          kind="ExternalInput").ap()
    ctx_d = nc.dram_tensor("context", [B_LOCAL, K, D], fp32,
                           kind="ExternalInput").ap()
    kern = nc.dram_tensor("kernel", [K, K], fp32, kind="ExternalInput").ap()
    bias = nc.dram_tensor("bias", [K], fp32, kind="ExternalInput").ap()
    out = nc.dram_tensor("out", [B_LOCAL, D], fp32, kind="ExternalOutput").ap()

    n_full = B_LOCAL // P          # 195 full tiles
    rem = B_LOCAL - n_full * P     # 40 leftover rows

    dist_v = dist[:n_full * P, :].rearrange("(n p) k -> p n k", p=P)
    ctx_v = ctx_d[:n_full * P].rearrange("(n p) k d -> p n (k d)", p=P)
    out_v = out[:n_full * P, :].rearrange("(n p) d -> p n d", p=P)

    with tile.TileContext(nc) as tc:
        from contextlib import ExitStack
        with ExitStack() as st:
            consts = st.enter_context(tc.tile_pool(name="consts", bufs=1))
            big = st.enter_context(tc.tile_pool(name="big", bufs=1))
            ctxp = st.enter_context(tc.tile_pool(name="ctx", bufs=3))
            prodp = st.enter_context(tc.tile_pool(name="prod", bufs=2))
            small = st.enter_context(tc.tile_pool(name="small", bufs=3))
            psum_t = st.enter_context(
                tc.tile_pool(name="psumT", bufs=2, space="PSUM"))
            psum_l = st.enter_context(
                tc.tile_pool(name="psumL", bufs=2, space="PSUM"))

            ident = consts.tile([P, P], fp32)
            make_identity(nc, ident)
            kern_s = consts.tile([K, K], fp32)
            nc.sync.dma_start(out=kern_s, in_=kern)
            bias_s = consts.tile([1, K], fp32)
            nc.sync.dma_start(out=bias_s, in_=bias.unsqueeze(0))
            ones_s = consts.tile([1, P], fp32)
            nc.vector.memset(ones_s, 1.0)

            # All distances for the full tiles; squared+exp'd in place.
            simi_all = big.tile([P, n_full, K], fp32)
            nc.sync.dma_start(out=simi_all, in_=dist_v)
            nc.scalar.activation(out=simi_all, in_=simi_all, func=AF.Square)
            nc.scalar.activation(out=simi_all, in_=simi_all, func=AF.Exp,
                                 scale=-0.5)

            # Staged output for the full tiles (one big DMA at the end).
            mean_all = big.tile([P, n_full, D], fp32)

            def softmax_weights(simi_ap, rows):
                """simi [rows, K] -> normalized weights wn [rows, K] (SBUF)."""
                simiT_p = psum_t.tile([K, P], fp32, tag="simiT_p")
                nc.tensor.transpose(out=simiT_p[:, :rows], in_=simi_ap,
                                    identity=ident[:rows, :rows])
                simiT_s = small.tile([K, P], fp32, tag="simiT_s")
                nc.scalar.copy(out=simiT_s[:, :rows], in_=simiT_p[:, :rows])

                logits_p = psum_l.tile([P, K], fp32, tag="logits_p")
                nc.tensor.matmul(out=logits_p[:rows, :],
                                 lhsT=simiT_s[:, :rows], rhs=kern_s,
                                 start=True, stop=False)
                nc.tensor.matmul(out=logits_p[:rows, :],
                                 lhsT=ones_s[:, :rows], rhs=bias_s,
                                 start=False, stop=True)

                expw = small.tile([P, K], fp32, tag="expw")
                zsum = small.tile([P, 1], fp32, tag="zsum")
                nc.scalar.activation(out=expw[:rows, :], in_=logits_p[:rows, :],
                                     func=AF.Exp, accum_out=zsum[:rows, :])
                rz = small.tile([P, 1], fp32, tag="rz")
                nc.vector.reciprocal(out=rz[:rows, :], in_=zsum[:rows, :])
                wn = small.tile([P, K], fp32, tag="wn")
                nc.vector.tensor_scalar_mul(out=wn[:rows, :],
                                            in0=expw[:rows, :],
                                            scalar1=rz[:rows, :])
                return wn

            # Full tiles: context DMA in CT-tile chunks, products in PT-tile
            # interleaved chunks with one GPSIMD fold each.
            ctx_tiles = {}

            for c0 in range(0, n_full, PT):
                pn = min(PT, n_full - c0)
                prod = prodp.tile([P, PT, DH, K, IL], fp32, tag="prod")
                for j in range(pn):
                    t = c0 + j
                    # context chunk load (every CT tiles)
                    cc, lane = divmod(t, CT)
                    if cc not in ctx_tiles:
                        ctx_tile = ctxp.tile([P, CT, K * D], fp32, tag="ctx")
                        lo = cc * CT
                        cn = min(CT, n_full - lo)
                        nc.sync.dma_start(out=ctx_tile[:, :cn, :],
                                          in_=ctx_v[:, lo:lo + cn, :])
                        ctx_tiles[cc] = ctx_tile
                    ctx3 = ctx_tiles[cc][:, lane, :].rearrange(
                        "p (k d) -> p k d", k=K)

                    wn = softmax_weights(simi_all[:, t, :], P)

                    # DVE: product for slabs [0, DVE_SLABS), interleaved out
                    nc.vector.tensor_mul(
                        out=prod[:, j].rearrange("p h k l -> p k h l")[:, :DVE_SLABS],
                        in0=ctx3[:, :DVE_SLABS, :],
                        in1=wn[:, :DVE_SLABS].unsqueeze(2).broadcast_to(
                            [P, DVE_SLABS, D]))
                    # ACT: product for the remaining slabs
                    for k in range(DVE_SLABS, K):
                        nc.scalar.mul(out=prod[:, j, :, k, :],
                                      in_=ctx3[:, k, :], mul=wn[:, k:k + 1])

                # GPSIMD: fold k -> k+KH pairs for the whole chunk
                nc.gpsimd.tensor_add(out=prod[:, :pn, :, 0:KH, :],
                                     in0=prod[:, :pn, :, 0:KH, :],
                                     in1=prod[:, :pn, :, KH:K, :])

                # DVE: reduce the folded KH slabs per tile
                for j in range(pn):
                    t = c0 + j
                    nc.vector.reduce_sum(
                        out=mean_all[:, t, :].rearrange("p (h l) -> p h l", l=IL),
                        in_=prod[:, j].rearrange("p h k l -> p h l k")[:, :, :, 0:KH],
                        axis=mybir.AxisListType.X)

            nc.sync.dma_start(out=out_v, in_=mean_all)

            # Remainder rows (partial tile): simple all-DVE path.
            if rem:
                simi_r = small.tile([P, K], fp32, tag="simi_r")
                nc.sync.dma_start(out=simi_r[:rem, :], in_=dist[n_full * P:, :])
                nc.scalar.activation(out=simi_r[:rem, :], in_=simi_r[:rem, :],
                                     func=AF.Square)
                nc.scalar.activation(out=simi_r[:rem, :], in_=simi_r[:rem, :],
                                     func=AF.Exp, scale=-0.5)
                ctx_r = ctxp.tile([P, CT, K * D], fp32, tag="ctx")
                nc.sync.dma_start(
                    out=ctx_r[:rem, 0, :],
                    in_=ctx_d[n_full * P:].rearrange("b k d -> b (k d)"))
                wn = softmax_weights(simi_r[:rem, :], rem)
                prod_r = small.tile([P, K, D], fp32, tag="prod_r")
                ctx3r = ctx_r[:rem, 0, :].rearrange("p (k d) -> p k d", k=K)
                nc.vector.tensor_mul(
                    out=prod_r[:rem], in0=ctx3r,
                    in1=wn[:rem, :].unsqueeze(2).broadcast_to([rem, K, D]))
                mean_r = small.tile([P, D], fp32, tag="mean_r")
                nc.vector.reduce_sum(
                    out=mean_r[:rem, :],
                    in_=prod_r[:rem].rearrange("p k d -> p d k"),
                    axis=mybir.AxisListType.X)
                nc.sync.dma_start(out=out[n_full * P:, :], in_=mean_r[:rem, :])

    nc.compile()
    return nc


def _get_nc():
    if "nc" not in _CACHE:
        _CACHE["nc"] = _build()
    return _CACHE["nc"]


def kernel(source_distance, context, kernel, bias, _trace=False, _tmpdir=None):
    from concourse.bass_utils import run_bass_kernel_spmd

    nc = _get_nc()

    source_distance = np.ascontiguousarray(source_distance, dtype=np.float32)
    context = np.ascontiguousarray(context, dtype=np.float32)
    kernel = np.ascontiguousarray(kernel, dtype=np.float32)
    bias = np.ascontiguousarray(bias, dtype=np.float32)

    in_maps = []
    for i in range(N_CORES):
        lo, hi = i * B_LOCAL, (i + 1) * B_LOCAL
        in_maps.append({
            "source_distance": source_distance[lo:hi],
            "context": context[lo:hi],
            "kernel": kernel,
            "bias": bias,
        })

    res = run_bass_kernel_spmd(nc, in_maps, list(range(N_CORES)),
                               trace=_trace, tmpdir=_tmpdir)
    out = np.concatenate([res.results[i]["out"] for i in range(N_CORES)], axis=0)
    if _trace:
        _CACHE["last_results"] = res
    return out
